# revision 37
# baseline (speedup 1.0000x reference)
"""GQA attention (dense_transformer) distributed over 8 TRN2 NeuronCores.

Sharding: batch (2) x head-groups (4). Core c = 4*b + g handles batch b,
q-heads 4g..4g+3 and kv-head g (GQA group local). Megatron-style:
 - QKV projection with column-sharded weights, x^T replicated per batch group
 - RoPE fused into the PSUM->SBUF eviction (host permutes wq/wk columns to
   [even dims; odd dims] per head so rotation is a partition-block affair);
   the final rotation combines run on gpsimd so the DVE keeps up
 - attention computed transposed (scoresT: k on partitions, q on free) so the
   AV matmul needs no transposes
 - causal: projection s-chunks are interleaved with attention rounds (round j
   only needs chunks <= j); the per-round AllGathers fire early and overlap
   later projection chunks; all out-projections run at the end so no PE-queue
   instruction ever waits on a collective while later PE work is ready
   (the queues are in-order - a stalled head blocks everything behind it)
 - softmax denominators: exp tiles are pair/quad-reduced on the DVE (bf16),
   then a short ones-matmul accumulation over the quad tiles; the causal
   diagonal bias is added by the DVE directly into the score PSUM
 - after each q-chunk j, a 4-rank AllGather shares attnT[:, chunk j] (all 16
   heads) with the batch group; every core then runs the out-projection for
   chunk j against ITS OWN 512-column slice of wo (a per-core host input, so
   the graph stays rank-independent)
 - output is written bf16 as (S, 512 cols per core); host concatenates
   column blocks and upcasts to f32.

DMA discipline: issue cost on the queue engines is per-DESCRIPTOR (~5ns), and
descriptors are contiguous runs. So every bulk operand is host-permuted so
that one SBUF partition's whole free range is a single contiguous DRAM run:
x chunks / wq / woc move as 128 descriptors of 16KB instead of 2048 of 1KB.
The attention output tiles are staged per round into one [128, 4*512] tile so
the bounce write is 128 x 4KB, and the bounce layout is p-major per rank so
the gather readback is 512 x 4KB per chunk. Total descriptor count drops
~7x, which un-serializes the Sync queue and frees ring bandwidth for the
AllGathers.

All matmul operands are bf16 (fp32 PSUM accumulation); softmax runs in fp32
on the scalar engine with a constant shift folded into the exp bias.
"""

import os
import numpy as np

B = 2
S = 2048
DIM = 2048
NH = 16
NKV = 4
HD = 128
NCORES = 8
QH = NH // NKV  # q heads per core (= per kv group)
SC = 512  # q-chunk / s-chunk size
NSC = S // SC  # 4
NKT = S // HD  # 16 k-tiles
WOC = 512  # out-proj columns per core
SCALE = 1.0 / float(np.sqrt(HD))
ESHIFT = 12.0  # constant shift inside exp; cancels in softmax
MASKVAL = -1e30

_cache = {}


def _n_ktiles(j: int, causal: bool) -> int:
    return 4 * (j + 1) if causal else NKT


def _build(mode: str):
    """Build + compile the SPMD graph. mode in {'causal', 'none', 'general'}."""
    import concourse.bass as bass
    import concourse.mybir as mybir
    import concourse.tile as tile
    from concourse import bacc
    from concourse.masks import make_identity

    causal = mode == "causal"
    general = mode == "general"
    f32 = mybir.dt.float32
    bf16 = mybir.dt.bfloat16

    bias_dve = os.environ.get("KOPT_BIAS_DVE", "1") == "1"
    rope_gps = os.environ.get("KOPT_ROPE_GPS", "0") == "1"
    sum_gps = os.environ.get("KOPT_SUM_GPS", "0") == "1"

    nc = bacc.Bacc("TRN2", target_bir_lowering=False, debug=False, num_devices=NCORES)

    # host-permuted layouts: one SBUF partition's free range = one contiguous
    # DRAM run (16KB descriptors)
    xt_e = nc.dram_tensor("xt", [NSC, 128, NKT * SC], bf16, kind="ExternalInput")
    wq_e = nc.dram_tensor("wq", [128, NKT * QH * HD], bf16, kind="ExternalInput")
    wk_e = nc.dram_tensor("wk", [128, NKT * HD], bf16, kind="ExternalInput")
    wv_e = nc.dram_tensor("wv", [128, NKT * HD], bf16, kind="ExternalInput")
    woc_e = nc.dram_tensor("woc", [128, NKT * WOC], bf16, kind="ExternalInput")
    cos_e = nc.dram_tensor("cosT", [HD, S], bf16, kind="ExternalInput")
    sin_e = nc.dram_tensor("sinT", [HD, S], bf16, kind="ExternalInput")
    if causal:
        biasd_e = nc.dram_tensor("biasd", [HD, 4 * SC], f32, kind="ExternalInput")
    if general:
        maskb_e = nc.dram_tensor("maskb", [S, S], f32, kind="ExternalInput")
    out_e = nc.dram_tensor("out", [NSC, 128, 4, WOC], bf16, kind="ExternalOutput")

    with tile.TileContext(nc) as tc:
        with (
            tc.tile_pool(name="res", bufs=1) as res,
            tc.tile_pool(name="wqkv", bufs=1) as wp,
            tc.tile_pool(name="xp", bufs=2) as xp,
            tc.tile_pool(name="p1t", bufs=2) as p1t,
            tc.tile_pool(name="vtp", bufs=1) as vtp,
            tc.tile_pool(name="p2", bufs=7) as p2,
            tc.tile_pool(name="p3", bufs=2) as p3,
            tc.tile_pool(name="sump", bufs=6) as sump,
            tc.tile_pool(name="mb", bufs=4) as mbp,
            tc.tile_pool(name="agt", bufs=2) as agp,
            tc.tile_pool(name="osb", bufs=2) as osbp,
            tc.tile_pool(name="dram", bufs=1, space="DRAM") as dram,
            tc.tile_pool(name="gp", bufs=2, space="PSUM") as gpp,
            tc.tile_pool(name="scp", bufs=4, space="PSUM") as scp,
            tc.tile_pool(name="avp", bufs=2, space="PSUM") as avp,
        ):
            # ---- resident tiles ----
            qT = [res.tile([HD, S], bf16, tag=f"qT{h}", name=f"qT{h}") for h in range(QH)]
            kT = res.tile([HD, S], bf16, tag="kT")
            V = res.tile([HD, S], bf16, tag="V")  # cols [128kc:+128] = V chunk kc
            cosT = res.tile([HD, S], bf16, tag="cosT")
            sinT = res.tile([HD, S], bf16, tag="sinT")
            ident = res.tile([HD, HD], f32, tag="ident")
            identr = res.tile([HD, HD], bf16, tag="identr")
            ones = res.tile([HD, HD], bf16, tag="ones")
            ebias = res.tile([128, 1], f32, tag="ebias")
            woc_t = res.tile([128, NKT * WOC], bf16, tag="woc")
            if causal:
                biasd = res.tile([HD, 4 * SC], f32, tag="biasd")
            # phase-1 weights, resident; [:, 512d+128h:+128] = head h, block d
            wq_all = wp.tile([128, NKT * QH * HD], bf16, tag="wq")
            wk_all = wp.tile([128, NKT * HD], bf16, tag="wk")
            wv_all = wp.tile([128, NKT * HD], bf16, tag="wv")
            vT = vtp.tile([HD, S], f32, tag="vT")

            # bounce buffers for the attnT AllGathers, p-major per rank:
            # bnc_in[j] = [128 p, 4 h, 512 q]; bnc_out[j] = [4 r, 128 p, 4 h, 512 q]
            bnc_in = dram.tile([NSC, 128, QH, SC], bf16)
            bnc_out = dram.tile([NSC, 4, 128, QH, SC], bf16)
            # round-3 per-head bounce (head-major) so each head's AG fires
            # the moment that head finishes
            bnc3_in = dram.tile([QH, 128, SC], bf16)
            bnc3_out = dram.tile([QH, 4, 128, SC], bf16)
            wup_in = dram.tile([128, 4], bf16)
            wup_out = dram.tile([512, 4], bf16)

            def ham_warmup():
                """Dummy matmuls on memset data during the startup DMA ramp:
                the PE's HAM clock gate needs ~3.4us of sustained activity to
                reach full clock, so without these the first ~16 real
                matmuls run at half rate."""
                wtile = res.tile([128, SC], bf16, tag="hamw", name="hamw")
                nc.vector.memset(wtile[:, :], 0.0)
                wps = scp.tile([128, SC], f32, tag="sc", name="hamps")
                for i in range(16):
                    nc.tensor.matmul(
                        wps[:, :],
                        lhsT=wtile[:, 0:128],
                        rhs=wtile[:, :],
                        start=(i == 0),
                        stop=(i == 15),
                    )

            def warmup_ag():
                """Tiny AllGather at kernel start: pays first-collective
                setup cost and aligns the ranks while proj0 computes."""
                wt = res.tile([128, 4], bf16, tag="wup", name="wup")
                nc.vector.memset(wt[:, :], 0.0)
                nc.sync.dma_start(out=wup_in[:, :], in_=wt[:, :])
                nc.gpsimd.collective_compute(
                    "AllGather",
                    bass.mybir.AluOpType.bypass,
                    replica_groups=[[0, 1, 2, 3], [4, 5, 6, 7]],
                    ins=[wup_in[:, :].opt()],
                    outs=[wup_out[:, :].opt()],
                )

            # per-round readback tiles of the gathered attnT chunk
            agt = {}

            def prelude_dmas():
                """Everything not needed in the first ~15us, issued after the
                first s-chunk's critical tiles so the pipeline starts early."""
                nc.vector.memset(ones[:, :], 1.0)
                if causal:
                    nc.sync.dma_start(out=biasd[:, :], in_=biasd_e[:, :])
                # cos/sin for chunks 1-3 in one transfer each
                rsl = slice(SC, S)
                nc.sync.dma_start(out=cosT[:, rsl], in_=cos_e[:, rsl])
                nc.sync.dma_start(out=sinT[:, rsl], in_=sin_e[:, rsl])
                make_identity(nc, ident[:, :])
                if not bias_dve or general:
                    # identr is only read by the matmul-bias diagonal path
                    nc.vector.tensor_copy(identr[:, :], ident[:, :])
                nc.vector.memset(ebias[:, :], -ESHIFT)

            def woc_dma():
                nc.sync.dma_start(out=woc_t[:, :], in_=woc_e[:, :])

            def rope_evict(psum, dst, sl):
                """dst[:, sl] = rotate(psum); cosT/sinT are [c;c]/[s;s]
                stacked. m2s holds the sin product with halves swapped so
                the combine steps see equal base partitions. The combines
                run on gpsimd (SBUF-only) to unload the DVE."""
                m1 = p1t.tile([128, SC], f32, tag="t1", name="m1")
                m2s = p1t.tile([128, SC], f32, tag="t2", name="m2s")
                nc.vector.tensor_mul(m1[:, :], psum[:, :], cosT[:, sl])
                nc.vector.tensor_mul(m2s[64:128, :], psum[0:64, :], sinT[0:64, sl])
                nc.vector.tensor_mul(m2s[0:64, :], psum[64:128, :], sinT[64:128, sl])
                eng = nc.gpsimd if rope_gps else nc.vector
                eng.tensor_sub(dst[0:64, sl], m1[0:64, :], m2s[0:64, :])
                eng.tensor_add(dst[64:128, sl], m1[64:128, :], m2s[64:128, :])

            def proj_sc(sc):
                """QKV projection + RoPE for s-chunk sc, plus V transposes of
                this chunk's four 128-column blocks. K chain runs first (its
                weights are the smallest transfer) so the PE starts early."""
                sl = slice(SC * sc, SC * sc + SC)
                xall = xp.tile([128, NKT * SC], bf16, tag="xp", name="xp")
                if sc == 0:
                    # chunk 0 split in quarters; wq (the largest weight)
                    # last - the K and V chains cover its transfer time
                    nc.sync.dma_start(out=xall[:, 0 : 4 * SC], in_=xt_e[0, :, 0 : 4 * SC])
                    nc.sync.dma_start(out=wk_all[:, 0 : 4 * HD], in_=wk_e[:, 0 : 4 * HD])
                    nc.sync.dma_start(out=wk_all[:, 4 * HD :], in_=wk_e[:, 4 * HD :])
                    nc.sync.dma_start(
                        out=xall[:, 4 * SC : 8 * SC], in_=xt_e[0, :, 4 * SC : 8 * SC]
                    )
                    nc.sync.dma_start(
                        out=xall[:, 8 * SC : 16 * SC], in_=xt_e[0, :, 8 * SC : 16 * SC]
                    )
                    nc.sync.dma_start(out=wv_all[:, :], in_=wv_e[:, :])
                    nc.sync.dma_start(out=cosT[:, sl], in_=cos_e[:, sl])
                    nc.sync.dma_start(out=sinT[:, sl], in_=sin_e[:, sl])
                    nc.sync.dma_start(out=wq_all[:, :], in_=wq_e[:, :])
                    prelude_dmas()
                else:
                    nc.sync.dma_start(out=xall[:, :], in_=xt_e[sc, :, :])

                def xs(d):
                    return xall[:, SC * d : SC * d + SC]

                # K and V first (their weights are the smallest transfers),
                # then the Q heads - covers the wq transfer at startup
                ps = gpp.tile([128, SC], f32, tag="gp")
                for d in range(NKT):
                    nc.tensor.matmul(
                        ps[:, :],
                        lhsT=wk_all[:, 128 * d : 128 * d + 128],
                        rhs=xs(d),
                        start=(d == 0),
                        stop=(d == NKT - 1),
                    )
                rope_evict(ps, kT, sl)
                ps = gpp.tile([128, SC], f32, tag="gp")
                for d in range(NKT):
                    nc.tensor.matmul(
                        ps[:, :],
                        lhsT=wv_all[:, 128 * d : 128 * d + 128],
                        rhs=xs(d),
                        start=(d == 0),
                        stop=(d == NKT - 1),
                    )
                nc.scalar.copy(vT[:, sl], ps[:, :])
                # transpose this chunk's vT columns -> V (shared score psum)
                for kc in range(4 * sc, 4 * sc + 4):
                    cs = slice(128 * kc, 128 * kc + 128)
                    pst = scp.tile([128, SC], f32, tag="sc", name="vtr")
                    nc.tensor.transpose(pst[:, 0:128], vT[:, cs], ident[:, :])
                    nc.scalar.copy(V[:, cs], pst[:, 0:128])
                for h in range(QH):
                    ps = gpp.tile([128, SC], f32, tag="gp")
                    for d in range(NKT):
                        nc.tensor.matmul(
                            ps[:, :],
                            lhsT=wq_all[:, SC * d + 128 * h : SC * d + 128 * h + 128],
                            rhs=xs(d),
                            start=(d == 0),
                            stop=(d == NKT - 1),
                        )
                    rope_evict(ps, qT[h], sl)

            def attn_round(j, hs, split=False):
                """Attention for q-chunk j, heads hs; at tiles staged into
                one [128, 4*512] tile, written to bnc_in[j] in ONE DMA.
                With split=True (last round) each head's tile is written and
                AllGathered individually so the final gather chain is one
                head deep instead of four."""
                qsl = slice(SC * j, SC * j + SC)
                nkt = _n_ktiles(j, causal)
                stage = p3.tile([HD, QH * SC], bf16, tag="stage", name=f"st{j}")
                for h in hs:
                    av_ps = avp.tile([HD, SC], f32, tag="av")
                    es = []  # exp tiles, then pair/quad reduced
                    for kt in range(nkt):
                        ks = slice(128 * kt, 128 * kt + 128)
                        sc_ps = scp.tile([128, SC], f32, tag="sc")
                        is_diag = causal and kt >= nkt - 4
                        use_mm_bias = (is_diag and not bias_dve) or general
                        nc.tensor.matmul(
                            sc_ps[:, :],
                            lhsT=kT[:, ks],
                            rhs=qT[h][:, qsl],
                            start=True,
                            stop=not use_mm_bias,
                        )
                        if is_diag and bias_dve:
                            di = kt - (nkt - 4)
                            nc.vector.tensor_add(
                                sc_ps[:, :], sc_ps[:, :], biasd[:, SC * di : SC * di + SC]
                            )
                        elif is_diag:
                            di = kt - (nkt - 4)
                            bb = mbp.tile([128, SC], bf16, tag="mb")
                            nc.vector.tensor_copy(bb[:, :], biasd[:, SC * di : SC * di + SC])
                            nc.tensor.matmul(
                                sc_ps[:, :],
                                lhsT=identr[:, :],
                                rhs=bb[:, :],
                                start=False,
                                stop=True,
                            )
                        elif general:
                            mbf = mbp.tile([128, SC], f32, tag="mbf")
                            nc.sync.dma_start(
                                out=mbf[:, :],
                                in_=maskb_e[128 * kt : 128 * kt + 128, qsl],
                            )
                            nc.vector.tensor_add(sc_ps[:, :], sc_ps[:, :], mbf[:, :])
                        e_sb = p2.tile([128, SC], bf16, tag="e")
                        nc.scalar.activation(
                            e_sb[:, :],
                            sc_ps[:, :],
                            mybir.ActivationFunctionType.Exp,
                            bias=ebias[:, :],
                            scale=SCALE,
                        )
                        nc.tensor.matmul(
                            av_ps[:, :],
                            lhsT=V[:, ks],
                            rhs=e_sb[:, :],
                            start=(kt == 0),
                            stop=(kt == nkt - 1),
                        )
                        es.append(e_sb)
                        if kt % 2 == 1:
                            ep = sump.tile([128, SC], bf16, tag="ep", name="ep")
                            nc.vector.tensor_add(ep[:, :], es[-2][:, :], es[-1][:, :])
                            es[-2:] = [ep]
                            if kt % 4 == 3:
                                eq = sump.tile([128, SC], bf16, tag="eq", name="eq")
                                eng = nc.gpsimd if sum_gps else nc.vector
                                eng.tensor_add(eq[:, :], es[-2][:, :], es[-1][:, :])
                                es[-2:] = [eq]
                    # es now holds nkt/4 quad tiles; sum over k via ones-matmul
                    sum_ps = scp.tile([128, SC], f32, tag="sc", name="sums")
                    for qi, eq in enumerate(es):
                        nc.tensor.matmul(
                            sum_ps[:, :],
                            lhsT=ones[:, :],
                            rhs=eq[:, :],
                            start=(qi == 0),
                            stop=(qi == len(es) - 1),
                        )
                    rec = p3.tile([128, SC], f32, tag="rec")
                    nc.vector.reciprocal_approx_fast(rec[:, :], sum_ps[:, :])
                    nc.vector.tensor_mul(
                        stage[:, SC * h : SC * h + SC], av_ps[:, :], rec[:, :]
                    )
                    if split:
                        nc.scalar.dma_start(
                            out=bnc3_in[h], in_=stage[:, SC * h : SC * h + SC]
                        )
                        ag_fire3(h)
                if not split:
                    nc.scalar.dma_start(out=bnc_in[j], in_=stage[:, :])

            def ag_fire3(h):
                """AllGather head h of the last round."""
                if os.environ.get("KOPT_NOCC", "0") == "1":
                    nc.sync.dma_start(out=bnc3_out[h, 0].opt(), in_=bnc3_in[h].opt())
                else:
                    nc.gpsimd.collective_compute(
                        "AllGather",
                        bass.mybir.AluOpType.bypass,
                        replica_groups=[[0, 1, 2, 3], [4, 5, 6, 7]],
                        ins=[bnc3_in[h].opt()],
                        outs=[bnc3_out[h].opt()],
                    )

            def readback3():
                """Per-head readbacks of the last round's gathers into one
                tile, free layout [h][r][q] (block for m-chunk cc=4r+h sits
                at 2048h+512r)."""
                t = agp.tile([128, 16 * SC], bf16, tag="ag", name="ag3")
                for h in range(QH):
                    nc.sync.dma_start(
                        out=t[:, 2048 * h : 2048 * h + 2048],
                        in_=bnc3_out[h].transpose((1, 0, 2)),
                    )
                agt[3] = t

            def outproj3():
                """Out-projection of the last chunk, accumulating per head
                as each head's gather lands. sts [0,1] use the gp PSUM pool,
                [2,3] the (tail-idle) av pool so all four row blocks stream
                concurrently."""
                ob = osbp.tile([128, 4 * WOC], bf16, tag="ob", name="ob3")
                pso = [gpp.tile([128, WOC], f32, tag="gp", name="op") for _ in range(2)]
                pso += [avp.tile([128, WOC], f32, tag="av", name="op") for _ in range(2)]
                for h in range(QH):
                    for st in range(4):
                        for r in range(4):
                            nc.tensor.matmul(
                                pso[st][:, :],
                                lhsT=agt[3][
                                    :, 2048 * h + SC * r + 128 * st : 2048 * h + SC * r + 128 * st + 128
                                ],
                                rhs=woc_t[:, SC * (4 * r + h) : SC * (4 * r + h) + SC],
                                start=(h == 0 and r == 0),
                                stop=(h == QH - 1 and r == 3),
                            )
                for st in range(4):
                    osl = slice(SC * st, SC * st + SC)
                    if st % 2 == 0:
                        nc.scalar.copy(ob[:, osl], pso[st][:, :])
                    else:
                        nc.vector.tensor_copy(ob[:, osl], pso[st][:, :])
                    if st == 1:
                        nc.scalar.dma_start(
                            out=out_e[3, :, 0:2], in_=ob[:, 0 : 2 * SC]
                        )
                nc.scalar.dma_start(out=out_e[3, :, 2:4], in_=ob[:, 2 * SC :])

            def ag_fire(j):
                """AllGather bnc_in chunk j to bnc_out[j]."""
                if os.environ.get("KOPT_NOCC", "0") == "1":
                    nc.sync.dma_start(out=bnc_out[j, 0].opt(), in_=bnc_in[j].opt())
                else:
                    nc.gpsimd.collective_compute(
                        "AllGather",
                        bass.mybir.AluOpType.bypass,
                        replica_groups=[[0, 1, 2, 3], [4, 5, 6, 7]],
                        ins=[bnc_in[j].opt()],
                        outs=[bnc_out[j].opt()],
                    )

            def readback(j):
                """Gathered chunk -> SBUF contraction tile, one DMA of 512 x
                4KB descriptors. Emitted on the Sync queue at points where
                AllGather j is already complete (or nothing later on the
                queue is urgent), so it fires the moment the collective
                lands."""
                t = agp.tile([128, 16 * SC], bf16, tag="ag", name=f"ag{j}")
                nc.sync.dma_start(out=t[:, :], in_=bnc_out[j].transpose((1, 0, 2, 3)))
                agt[j] = t

            def outproj_pair(j, sts, ob, start=True, stop=True, pso=None):
                """Accumulate out rows [512j + 128st] (this core's 512
                columns) for the two q-row blocks in sts, contracting over
                the 16 m-chunks of the gathered attnT chunk j."""
                if pso is None:
                    pso = [gpp.tile([128, WOC], f32, tag="gp", name="op") for _ in range(2)]
                for sti, st in enumerate(sts):
                    for cc in range(NKT):
                        nc.tensor.matmul(
                            pso[sti][:, :],
                            lhsT=agt[j][:, SC * cc + 128 * st : SC * cc + 128 * st + 128],
                            rhs=woc_t[:, SC * cc : SC * cc + SC],
                            start=(start and cc == 0),
                            stop=(stop and cc == NKT - 1),
                        )
                if stop:
                    for sti, st in enumerate(sts):
                        osl = slice(SC * st, SC * st + SC)
                        if sti % 2 == 0:
                            nc.scalar.copy(ob[:, osl], pso[sti][:, :])
                        else:
                            nc.vector.tensor_copy(ob[:, osl], pso[sti][:, :])
                return pso

            def outproj(j):
                ob = osbp.tile([128, 4 * WOC], bf16, tag="ob", name=f"ob{j}")
                outproj_pair(j, [0, 1], ob)
                outproj_pair(j, [2, 3], ob)
                nc.scalar.dma_start(out=out_e[j], in_=ob[:, :])

            # ---- schedule ----
            # out-projections are deferred to the end: the PE queue is
            # in-order, so an outproj emitted mid-stream would head-of-line
            # block later projection/attention matmuls whenever its
            # AllGather+readback hasn't landed yet.
            if os.environ.get("KOPT_WARMUP_AG", "1") == "1":
                warmup_ag()
            if os.environ.get("KOPT_HAM_WARMUP", "0") == "1":
                # warms the PE clock gate, but the gate re-demotes during
                # the (variable-length) DMA ramp before the first real
                # chain, so this is net-neutral at best; kept for reference
                ham_warmup()
            if causal:
                proj_sc(0)
                attn_round(0, range(QH))
                ag_fire(0)
                proj_sc(1)
                woc_dma()
                attn_round(1, range(QH))
                ag_fire(1)
                proj_sc(2)
                readback(0)
                attn_round(2, range(QH))
                ag_fire(2)
                proj_sc(3)
                readback(1)
                attn_round(3, range(QH), split=True)
                outproj(0)
                readback(2)
                outproj(1)
                outproj(2)
                readback3()
                outproj3()
            else:
                for sc in range(NSC):
                    proj_sc(sc)
                    if sc == 1:
                        woc_dma()
                attn_round(0, range(QH))
                ag_fire(0)
                attn_round(1, range(QH))
                ag_fire(1)
                readback(0)
                attn_round(2, range(QH))
                ag_fire(2)
                readback(1)
                attn_round(3, range(QH), split=True)
                outproj(0)
                readback(2)
                outproj(1)
                outproj(2)
                readback3()
                outproj3()

    nc.compile()
    return nc


def _perm_cols(w: np.ndarray, heads: list) -> np.ndarray:
    """Reorder head columns to [even dims; odd dims] for block RoPE."""
    cols = []
    for h in heads:
        base = HD * h
        cols.extend([base + 2 * i for i in range(HD // 2)])
        cols.extend([base + 2 * i + 1 for i in range(HD // 2)])
    return np.ascontiguousarray(w[:, cols])


def _pmajor(w: np.ndarray) -> np.ndarray:
    """[2048, N] weight -> [128, 16*N]: one partition's free range is one
    contiguous DRAM run (16 d-blocks side by side)."""
    n = w.shape[1]
    return np.ascontiguousarray(w.reshape(NKT, 128, n).transpose(1, 0, 2)).reshape(
        128, NKT * n
    )


def kernel(x, wq, wk, wv, wo, freqs_cos, freqs_sin, mask):
    from concourse.bass_utils import run_bass_kernel_spmd

    x = np.asarray(x, dtype=np.float32)
    wq = np.asarray(wq, dtype=np.float32)
    wk = np.asarray(wk, dtype=np.float32)
    wv = np.asarray(wv, dtype=np.float32)
    wo = np.asarray(wo, dtype=np.float32)
    freqs_cos = np.asarray(freqs_cos, dtype=np.float32)
    freqs_sin = np.asarray(freqs_sin, dtype=np.float32)
    mask = np.asarray(mask)

    if not mask.any():
        mode = "none"
    elif np.array_equal(mask, np.triu(np.ones((S, S), dtype=bool), k=1)):
        mode = "causal"
    else:
        mode = "general"

    if mode not in _cache:
        import time as _t

        t0 = _t.time()
        _cache[mode] = _build(mode)
        print(f"[kernel] built mode={mode} in {_t.time() - t0:.1f}s", flush=True)
    nc = _cache[mode]

    # ---- host-side prep (sharding + layout) ----
    import ml_dtypes

    # x chunk layout [chunk, p, d-block, s]: per-partition-contiguous
    xt = [
        np.ascontiguousarray(
            np.asarray(x[b].T, dtype=ml_dtypes.bfloat16)
            .reshape(NKT, 128, NSC, SC)
            .transpose(2, 1, 0, 3)
        ).reshape(NSC, 128, NKT * SC)
        for b in range(B)
    ]
    wo_r = wo.astype(ml_dtypes.bfloat16)
    cosT = np.ascontiguousarray(
        np.concatenate([freqs_cos.T, freqs_cos.T], axis=0)
    ).astype(ml_dtypes.bfloat16)
    sinT = np.ascontiguousarray(
        np.concatenate([freqs_sin.T, freqs_sin.T], axis=0)
    ).astype(ml_dtypes.bfloat16)

    if mode == "causal":
        # 4 diag patterns (delta = 0,128,256,384) packed as (128, 2048):
        # bias[i, 512*di + jq] = MASKVAL if (128*di + i) > jq else 0
        i_ = np.arange(HD)[:, None]
        jq = np.arange(SC)[None, :]
        biasd = np.concatenate(
            [
                np.where(128 * di + i_ > jq, np.float32(MASKVAL), np.float32(0.0))
                for di in range(4)
            ],
            axis=1,
        ).astype(np.float32)
    if mode == "general":
        maskb = np.ascontiguousarray(
            np.where(mask.T, np.float32(MASKVAL), np.float32(0.0))
        ).astype(np.float32)

    in_maps = []
    for core in range(NCORES):
        b, g = divmod(core, 4)
        heads = [QH * g + h for h in range(QH)]
        m = {
            "xt": xt[b],
            "wq": _pmajor(_perm_cols(wq, heads)).astype(ml_dtypes.bfloat16),
            "wk": _pmajor(_perm_cols(wk, [g])).astype(ml_dtypes.bfloat16),
            "wv": _pmajor(np.ascontiguousarray(wv[:, HD * g : HD * g + HD])).astype(
                ml_dtypes.bfloat16
            ),
            "woc": _pmajor(
                np.ascontiguousarray(wo_r[:, WOC * g : WOC * g + WOC])
            ),
            "cosT": cosT,
            "sinT": sinT,
        }
        if mode == "causal":
            m["biasd"] = biasd
        if mode == "general":
            m["maskb"] = maskb
        in_maps.append(m)

    import time as _t

    t0 = _t.time()
    print("[kernel] launching SPMD run", flush=True)
    res = run_bass_kernel_spmd(nc, in_maps, core_ids=list(range(NCORES)))
    print(f"[kernel] SPMD run done in {_t.time() - t0:.1f}s", flush=True)
    kernel._last_result = res

    out = np.empty((B, S, DIM), dtype=np.float32)
    for core in range(NCORES):
        b, g = divmod(core, 4)
        # out_e [j, p, st, c] -> rows 512j+128st+p
        o = res.results[core]["out"].reshape(NSC, 128, 4, WOC)
        out[b, :, WOC * g : WOC * g + WOC] = (
            o.transpose(0, 2, 1, 3).reshape(S, WOC).astype(np.float32)
        )
    return out


# revision 38
# speedup vs baseline: 1.0371x; 1.0371x over previous
"""GQA attention (dense_transformer) distributed over 8 TRN2 NeuronCores.

Sharding: batch (2) x head-groups (4). Core c = 4*b + g handles batch b,
q-heads 4g..4g+3 and kv-head g (GQA group local). Megatron-style:
 - QKV projection with column-sharded weights, x^T replicated per batch group
 - RoPE fused into the PSUM->SBUF eviction (host permutes wq/wk columns to
   [even dims; odd dims] per head so rotation is a partition-block affair);
   the final rotation combines run on gpsimd so the DVE keeps up
 - attention computed transposed (scoresT: k on partitions, q on free) so the
   AV matmul needs no transposes
 - causal: projection s-chunks are interleaved with attention rounds (round j
   only needs chunks <= j); the per-round AllGathers fire early and overlap
   later projection chunks; all out-projections run at the end so no PE-queue
   instruction ever waits on a collective while later PE work is ready
   (the queues are in-order - a stalled head blocks everything behind it)
 - softmax denominators: exp tiles are pair/quad-reduced on the DVE (bf16),
   then a short ones-matmul accumulation over the quad tiles; the causal
   diagonal bias is added by the DVE directly into the score PSUM
 - after each q-chunk j, a 4-rank AllGather shares attnT[:, chunk j] (all 16
   heads) with the batch group; every core then runs the out-projection for
   chunk j against ITS OWN 512-column slice of wo (a per-core host input, so
   the graph stays rank-independent)
 - output is written bf16 as (S, 512 cols per core); host concatenates
   column blocks and upcasts to f32.

DMA discipline: issue cost on the queue engines is per-DESCRIPTOR (~5ns), and
descriptors are contiguous runs. So every bulk operand is host-permuted so
that one SBUF partition's whole free range is a single contiguous DRAM run:
x chunks / wq / woc move as 128 descriptors of 16KB instead of 2048 of 1KB.
The attention output tiles are staged per round into one [128, 4*512] tile so
the bounce write is 128 x 4KB, and the bounce layout is p-major per rank so
the gather readback is 512 x 4KB per chunk. Total descriptor count drops
~7x, which un-serializes the Sync queue and frees ring bandwidth for the
AllGathers.

All matmul operands are bf16 (fp32 PSUM accumulation); softmax runs in fp32
on the scalar engine with a constant shift folded into the exp bias.
"""

import os
import numpy as np

B = 2
S = 2048
DIM = 2048
NH = 16
NKV = 4
HD = 128
NCORES = 8
QH = NH // NKV  # q heads per core (= per kv group)
SC = 512  # q-chunk / s-chunk size
NSC = S // SC  # 4
NKT = S // HD  # 16 k-tiles
WOC = 512  # out-proj columns per core
SCALE = 1.0 / float(np.sqrt(HD))
ESHIFT = 12.0  # constant shift inside exp; cancels in softmax
MASKVAL = -1e30

_cache = {}


def _n_ktiles(j: int, causal: bool) -> int:
    return 4 * (j + 1) if causal else NKT


def _build(mode: str):
    """Build + compile the SPMD graph. mode in {'causal', 'none', 'general'}."""
    import concourse.bass as bass
    import concourse.mybir as mybir
    import concourse.tile as tile
    from concourse import bacc
    from concourse.masks import make_identity

    causal = mode == "causal"
    general = mode == "general"
    f32 = mybir.dt.float32
    bf16 = mybir.dt.bfloat16

    bias_dve = os.environ.get("KOPT_BIAS_DVE", "1") == "1"
    rope_gps = os.environ.get("KOPT_ROPE_GPS", "0") == "1"
    sum_gps = os.environ.get("KOPT_SUM_GPS", "0") == "1"

    nc = bacc.Bacc("TRN2", target_bir_lowering=False, debug=False, num_devices=NCORES)

    # host-permuted layouts: one SBUF partition's free range = one contiguous
    # DRAM run (16KB descriptors)
    xt_e = nc.dram_tensor("xt", [NSC, 128, NKT * SC], bf16, kind="ExternalInput")
    wq_e = nc.dram_tensor("wq", [128, NKT * QH * HD], bf16, kind="ExternalInput")
    wk_e = nc.dram_tensor("wk", [128, NKT * HD], bf16, kind="ExternalInput")
    wv_e = nc.dram_tensor("wv", [128, NKT * HD], bf16, kind="ExternalInput")
    woc_e = nc.dram_tensor("woc", [128, NKT * WOC], bf16, kind="ExternalInput")
    cos_e = nc.dram_tensor("cosT", [HD, S], bf16, kind="ExternalInput")
    sin_e = nc.dram_tensor("sinT", [HD, S], bf16, kind="ExternalInput")
    if causal:
        biasd_e = nc.dram_tensor("biasd", [HD, 4 * SC], f32, kind="ExternalInput")
    if general:
        maskb_e = nc.dram_tensor("maskb", [S, S], f32, kind="ExternalInput")
    out_e = nc.dram_tensor("out", [NSC, 128, 4, WOC], bf16, kind="ExternalOutput")

    with tile.TileContext(nc) as tc:
        with (
            tc.tile_pool(name="res", bufs=1) as res,
            tc.tile_pool(name="wqkv", bufs=1) as wp,
            tc.tile_pool(name="xp", bufs=2) as xp,
            tc.tile_pool(name="p1t", bufs=2) as p1t,
            tc.tile_pool(name="vtp", bufs=1) as vtp,
            tc.tile_pool(name="p2", bufs=6) as p2,
            tc.tile_pool(name="p3", bufs=2) as p3,
            tc.tile_pool(name="sump", bufs=5) as sump,
            tc.tile_pool(name="mb", bufs=4) as mbp,
            tc.tile_pool(name="agt", bufs=2) as agp,
            tc.tile_pool(name="osb", bufs=2) as osbp,
            tc.tile_pool(name="dram", bufs=1, space="DRAM") as dram,
            tc.tile_pool(name="gp", bufs=2, space="PSUM") as gpp,
            tc.tile_pool(name="scp", bufs=4, space="PSUM") as scp,
            tc.tile_pool(name="avp", bufs=2, space="PSUM") as avp,
        ):
            # ---- resident tiles ----
            qT = [res.tile([HD, S], bf16, tag=f"qT{h}", name=f"qT{h}") for h in range(QH)]
            kT = res.tile([HD, S], bf16, tag="kT")
            V = res.tile([HD, S], bf16, tag="V")  # cols [128kc:+128] = V chunk kc
            cosT = res.tile([HD, S], bf16, tag="cosT")
            sinT = res.tile([HD, S], bf16, tag="sinT")
            ident = res.tile([HD, HD], f32, tag="ident")
            identr = res.tile([HD, HD], bf16, tag="identr")
            ones = res.tile([HD, HD], bf16, tag="ones")
            ebias = res.tile([128, 1], f32, tag="ebias")
            woc_t = res.tile([128, NKT * WOC], bf16, tag="woc")
            if causal:
                biasd = res.tile([HD, 4 * SC], f32, tag="biasd")
            # phase-1 weights, resident; [:, 512d+128h:+128] = head h, block d
            wq_all = wp.tile([128, NKT * QH * HD], bf16, tag="wq")
            wk_all = wp.tile([128, NKT * HD], bf16, tag="wk")
            wv_all = wp.tile([128, NKT * HD], bf16, tag="wv")
            vT = vtp.tile([HD, S], f32, tag="vT")

            # bounce buffers for the attnT AllGathers, p-major per rank:
            # bnc_in[j] = [128 p, 4 h, 512 q]; bnc_out[j] = [4 r, 128 p, 4 h, 512 q]
            bnc_in = dram.tile([NSC, 128, QH, SC], bf16)
            bnc_out = dram.tile([NSC, 4, 128, QH, SC], bf16)
            # round-3 per-head bounce (head-major) so each head's AG fires
            # the moment that head finishes
            bnc3_in = dram.tile([QH, 128, SC], bf16)
            bnc3_out = dram.tile([QH, 4, 128, SC], bf16)
            wup_in = dram.tile([128, 4], bf16)
            wup_out = dram.tile([512, 4], bf16)

            def ham_warmup():
                """Dummy matmuls on memset data during the startup DMA ramp:
                the PE's HAM clock gate needs ~3.4us of sustained activity to
                reach full clock, so without these the first ~16 real
                matmuls run at half rate."""
                wtile = res.tile([128, SC], bf16, tag="hamw", name="hamw")
                nc.vector.memset(wtile[:, :], 0.0)
                wps = scp.tile([128, SC], f32, tag="sc", name="hamps")
                for i in range(16):
                    nc.tensor.matmul(
                        wps[:, :],
                        lhsT=wtile[:, 0:128],
                        rhs=wtile[:, :],
                        start=(i == 0),
                        stop=(i == 15),
                    )

            def warmup_ag():
                """Tiny AllGather at kernel start: pays first-collective
                setup cost and aligns the ranks while proj0 computes."""
                wt = res.tile([128, 4], bf16, tag="wup", name="wup")
                nc.vector.memset(wt[:, :], 0.0)
                nc.sync.dma_start(out=wup_in[:, :], in_=wt[:, :])
                nc.gpsimd.collective_compute(
                    "AllGather",
                    bass.mybir.AluOpType.bypass,
                    replica_groups=[[0, 1, 2, 3], [4, 5, 6, 7]],
                    ins=[wup_in[:, :].opt()],
                    outs=[wup_out[:, :].opt()],
                )

            # per-round readback tiles of the gathered attnT chunk
            agt = {}

            def prelude_dmas():
                """Everything not needed in the first ~15us, issued after the
                first s-chunk's critical tiles so the pipeline starts early."""
                nc.vector.memset(ones[:, :], 1.0)
                if causal:
                    nc.sync.dma_start(out=biasd[:, :], in_=biasd_e[:, :])
                # cos/sin for chunks 1-3 in one transfer each
                rsl = slice(SC, S)
                nc.sync.dma_start(out=cosT[:, rsl], in_=cos_e[:, rsl])
                nc.sync.dma_start(out=sinT[:, rsl], in_=sin_e[:, rsl])
                make_identity(nc, ident[:, :])
                if not bias_dve or general:
                    # identr is only read by the matmul-bias diagonal path
                    nc.vector.tensor_copy(identr[:, :], ident[:, :])
                nc.vector.memset(ebias[:, :], -ESHIFT)

            def woc_dma():
                nc.sync.dma_start(out=woc_t[:, :], in_=woc_e[:, :])

            def rope_evict(psum, dst, sl):
                """dst[:, sl] = rotate(psum); cosT/sinT are [c;c]/[s;s]
                stacked. m2s holds the sin product with halves swapped so
                the combine steps see equal base partitions. The combines
                run on gpsimd (SBUF-only) to unload the DVE."""
                m1 = p1t.tile([128, SC], f32, tag="t1", name="m1")
                m2s = p1t.tile([128, SC], f32, tag="t2", name="m2s")
                nc.vector.tensor_mul(m1[:, :], psum[:, :], cosT[:, sl])
                nc.vector.tensor_mul(m2s[64:128, :], psum[0:64, :], sinT[0:64, sl])
                nc.vector.tensor_mul(m2s[0:64, :], psum[64:128, :], sinT[64:128, sl])
                eng = nc.gpsimd if rope_gps else nc.vector
                eng.tensor_sub(dst[0:64, sl], m1[0:64, :], m2s[0:64, :])
                eng.tensor_add(dst[64:128, sl], m1[64:128, :], m2s[64:128, :])

            def proj_sc(sc):
                """QKV projection + RoPE for s-chunk sc, plus V transposes of
                this chunk's four 128-column blocks. K chain runs first (its
                weights are the smallest transfer) so the PE starts early."""
                sl = slice(SC * sc, SC * sc + SC)
                xall = xp.tile([128, NKT * SC], bf16, tag="xp", name="xp")
                if sc == 0:
                    # chunk 0 split in quarters; wq (the largest weight)
                    # last - the K and V chains cover its transfer time
                    nc.sync.dma_start(out=xall[:, 0 : 4 * SC], in_=xt_e[0, :, 0 : 4 * SC])
                    nc.sync.dma_start(out=wk_all[:, 0 : 4 * HD], in_=wk_e[:, 0 : 4 * HD])
                    nc.sync.dma_start(out=wk_all[:, 4 * HD :], in_=wk_e[:, 4 * HD :])
                    nc.sync.dma_start(
                        out=xall[:, 4 * SC : 8 * SC], in_=xt_e[0, :, 4 * SC : 8 * SC]
                    )
                    nc.sync.dma_start(
                        out=xall[:, 8 * SC : 16 * SC], in_=xt_e[0, :, 8 * SC : 16 * SC]
                    )
                    nc.sync.dma_start(out=wv_all[:, :], in_=wv_e[:, :])
                    nc.sync.dma_start(out=cosT[:, sl], in_=cos_e[:, sl])
                    nc.sync.dma_start(out=sinT[:, sl], in_=sin_e[:, sl])
                    nc.sync.dma_start(out=wq_all[:, :], in_=wq_e[:, :])
                    prelude_dmas()
                else:
                    nc.sync.dma_start(out=xall[:, :], in_=xt_e[sc, :, :])

                def xs(d):
                    return xall[:, SC * d : SC * d + SC]

                # K and V first (their weights are the smallest transfers),
                # then the Q heads - covers the wq transfer at startup
                ps = gpp.tile([128, SC], f32, tag="gp")
                for d in range(NKT):
                    nc.tensor.matmul(
                        ps[:, :],
                        lhsT=wk_all[:, 128 * d : 128 * d + 128],
                        rhs=xs(d),
                        start=(d == 0),
                        stop=(d == NKT - 1),
                    )
                rope_evict(ps, kT, sl)
                ps = gpp.tile([128, SC], f32, tag="gp")
                for d in range(NKT):
                    nc.tensor.matmul(
                        ps[:, :],
                        lhsT=wv_all[:, 128 * d : 128 * d + 128],
                        rhs=xs(d),
                        start=(d == 0),
                        stop=(d == NKT - 1),
                    )
                nc.scalar.copy(vT[:, sl], ps[:, :])
                # transpose this chunk's vT columns -> V (shared score psum)
                for kc in range(4 * sc, 4 * sc + 4):
                    cs = slice(128 * kc, 128 * kc + 128)
                    pst = scp.tile([128, SC], f32, tag="sc", name="vtr")
                    nc.tensor.transpose(pst[:, 0:128], vT[:, cs], ident[:, :])
                    nc.scalar.copy(V[:, cs], pst[:, 0:128])
                for h in range(QH):
                    ps = gpp.tile([128, SC], f32, tag="gp")
                    for d in range(NKT):
                        nc.tensor.matmul(
                            ps[:, :],
                            lhsT=wq_all[:, SC * d + 128 * h : SC * d + 128 * h + 128],
                            rhs=xs(d),
                            start=(d == 0),
                            stop=(d == NKT - 1),
                        )
                    rope_evict(ps, qT[h], sl)

            def attn_round(j, hs, split=False):
                """Attention for q-chunk j, heads hs; at tiles staged into
                one [128, 4*512] tile, written to bnc_in[j] in ONE DMA.
                With split=True (last round) each head's tile is written and
                AllGathered individually so the final gather chain is one
                head deep instead of four."""
                qsl = slice(SC * j, SC * j + SC)
                nkt = _n_ktiles(j, causal)
                stage = p3.tile([HD, QH * SC], bf16, tag="stage", name=f"st{j}")
                for h in hs:
                    av_ps = avp.tile([HD, SC], f32, tag="av")
                    es = []  # exp tiles, then pair/quad reduced
                    for kt in range(nkt):
                        ks = slice(128 * kt, 128 * kt + 128)
                        sc_ps = scp.tile([128, SC], f32, tag="sc")
                        is_diag = causal and kt >= nkt - 4
                        use_mm_bias = (is_diag and not bias_dve) or general
                        nc.tensor.matmul(
                            sc_ps[:, :],
                            lhsT=kT[:, ks],
                            rhs=qT[h][:, qsl],
                            start=True,
                            stop=not use_mm_bias,
                        )
                        if is_diag and bias_dve:
                            di = kt - (nkt - 4)
                            nc.vector.tensor_add(
                                sc_ps[:, :], sc_ps[:, :], biasd[:, SC * di : SC * di + SC]
                            )
                        elif is_diag:
                            di = kt - (nkt - 4)
                            bb = mbp.tile([128, SC], bf16, tag="mb")
                            nc.vector.tensor_copy(bb[:, :], biasd[:, SC * di : SC * di + SC])
                            nc.tensor.matmul(
                                sc_ps[:, :],
                                lhsT=identr[:, :],
                                rhs=bb[:, :],
                                start=False,
                                stop=True,
                            )
                        elif general:
                            mbf = mbp.tile([128, SC], f32, tag="mbf")
                            nc.sync.dma_start(
                                out=mbf[:, :],
                                in_=maskb_e[128 * kt : 128 * kt + 128, qsl],
                            )
                            nc.vector.tensor_add(sc_ps[:, :], sc_ps[:, :], mbf[:, :])
                        e_sb = p2.tile([128, SC], bf16, tag="e")
                        nc.scalar.activation(
                            e_sb[:, :],
                            sc_ps[:, :],
                            mybir.ActivationFunctionType.Exp,
                            bias=ebias[:, :],
                            scale=SCALE,
                        )
                        nc.tensor.matmul(
                            av_ps[:, :],
                            lhsT=V[:, ks],
                            rhs=e_sb[:, :],
                            start=(kt == 0),
                            stop=(kt == nkt - 1),
                        )
                        es.append(e_sb)
                        if kt % 2 == 1:
                            ep = sump.tile([128, SC], bf16, tag="ep", name="ep")
                            nc.vector.tensor_add(ep[:, :], es[-2][:, :], es[-1][:, :])
                            es[-2:] = [ep]
                            if kt % 4 == 3:
                                eq = sump.tile([128, SC], bf16, tag="eq", name="eq")
                                eng = nc.gpsimd if sum_gps else nc.vector
                                eng.tensor_add(eq[:, :], es[-2][:, :], es[-1][:, :])
                                es[-2:] = [eq]
                    # es now holds nkt/4 quad tiles; sum over k via ones-matmul
                    sum_ps = scp.tile([128, SC], f32, tag="sc", name="sums")
                    for qi, eq in enumerate(es):
                        nc.tensor.matmul(
                            sum_ps[:, :],
                            lhsT=ones[:, :],
                            rhs=eq[:, :],
                            start=(qi == 0),
                            stop=(qi == len(es) - 1),
                        )
                    rec = p3.tile([128, SC], f32, tag="rec")
                    nc.vector.reciprocal_approx_fast(rec[:, :], sum_ps[:, :])
                    nc.vector.tensor_mul(
                        stage[:, SC * h : SC * h + SC], av_ps[:, :], rec[:, :]
                    )
                    if split:
                        nc.scalar.dma_start(
                            out=bnc3_in[h], in_=stage[:, SC * h : SC * h + SC]
                        )
                        ag_fire3(h)
                if not split:
                    nc.scalar.dma_start(out=bnc_in[j], in_=stage[:, :])

            def ag_fire3(h):
                """AllGather head h of the last round."""
                if os.environ.get("KOPT_NOCC", "0") == "1":
                    nc.sync.dma_start(out=bnc3_out[h, 0].opt(), in_=bnc3_in[h].opt())
                else:
                    nc.gpsimd.collective_compute(
                        "AllGather",
                        bass.mybir.AluOpType.bypass,
                        replica_groups=[[0, 1, 2, 3], [4, 5, 6, 7]],
                        ins=[bnc3_in[h].opt()],
                        outs=[bnc3_out[h].opt()],
                    )

            def readback3():
                """Per-head readbacks of the last round's gathers into one
                tile, free layout [h][r][q] (block for m-chunk cc=4r+h sits
                at 2048h+512r)."""
                t = agp.tile([128, 16 * SC], bf16, tag="ag", name="ag3")
                for h in range(QH):
                    nc.sync.dma_start(
                        out=t[:, 2048 * h : 2048 * h + 2048],
                        in_=bnc3_out[h].transpose((1, 0, 2)),
                    )
                agt[3] = t

            def outproj3():
                """Out-projection of the last chunk, accumulating per head
                as each head's gather lands. sts [0,1] use the gp PSUM pool,
                [2,3] the (tail-idle) av pool so all four row blocks stream
                concurrently."""
                ob = osbp.tile([128, 4 * WOC], bf16, tag="ob", name="ob3")
                pso = [gpp.tile([128, WOC], f32, tag="gp", name="op") for _ in range(2)]
                pso += [avp.tile([128, WOC], f32, tag="av", name="op") for _ in range(2)]
                for h in range(QH):
                    for st in range(4):
                        for r in range(4):
                            nc.tensor.matmul(
                                pso[st][:, :],
                                lhsT=agt[3][
                                    :, 2048 * h + SC * r + 128 * st : 2048 * h + SC * r + 128 * st + 128
                                ],
                                rhs=woc_t[:, SC * (4 * r + h) : SC * (4 * r + h) + SC],
                                start=(h == 0 and r == 0),
                                stop=(h == QH - 1 and r == 3),
                            )
                for st in range(4):
                    osl = slice(SC * st, SC * st + SC)
                    if st % 2 == 0:
                        nc.scalar.copy(ob[:, osl], pso[st][:, :])
                    else:
                        nc.vector.tensor_copy(ob[:, osl], pso[st][:, :])
                    if st == 1:
                        nc.scalar.dma_start(
                            out=out_e[3, :, 0:2], in_=ob[:, 0 : 2 * SC]
                        )
                nc.scalar.dma_start(out=out_e[3, :, 2:4], in_=ob[:, 2 * SC :])

            def ag_fire(j):
                """AllGather bnc_in chunk j to bnc_out[j]."""
                if os.environ.get("KOPT_NOCC", "0") == "1":
                    nc.sync.dma_start(out=bnc_out[j, 0].opt(), in_=bnc_in[j].opt())
                else:
                    nc.gpsimd.collective_compute(
                        "AllGather",
                        bass.mybir.AluOpType.bypass,
                        replica_groups=[[0, 1, 2, 3], [4, 5, 6, 7]],
                        ins=[bnc_in[j].opt()],
                        outs=[bnc_out[j].opt()],
                    )

            def readback(j):
                """Gathered chunk -> SBUF contraction tile, one DMA of 512 x
                4KB descriptors. Emitted on the Sync queue at points where
                AllGather j is already complete (or nothing later on the
                queue is urgent), so it fires the moment the collective
                lands."""
                t = agp.tile([128, 16 * SC], bf16, tag="ag", name=f"ag{j}")
                nc.sync.dma_start(out=t[:, :], in_=bnc_out[j].transpose((1, 0, 2, 3)))
                agt[j] = t

            def outproj_pair(j, sts, ob, start=True, stop=True, pso=None):
                """Accumulate out rows [512j + 128st] (this core's 512
                columns) for the two q-row blocks in sts, contracting over
                the 16 m-chunks of the gathered attnT chunk j."""
                if pso is None:
                    pso = [gpp.tile([128, WOC], f32, tag="gp", name="op") for _ in range(2)]
                for sti, st in enumerate(sts):
                    for cc in range(NKT):
                        nc.tensor.matmul(
                            pso[sti][:, :],
                            lhsT=agt[j][:, SC * cc + 128 * st : SC * cc + 128 * st + 128],
                            rhs=woc_t[:, SC * cc : SC * cc + SC],
                            start=(start and cc == 0),
                            stop=(stop and cc == NKT - 1),
                        )
                if stop:
                    for sti, st in enumerate(sts):
                        osl = slice(SC * st, SC * st + SC)
                        if sti % 2 == 0:
                            nc.scalar.copy(ob[:, osl], pso[sti][:, :])
                        else:
                            nc.vector.tensor_copy(ob[:, osl], pso[sti][:, :])
                return pso

            def outproj(j):
                ob = osbp.tile([128, 4 * WOC], bf16, tag="ob", name=f"ob{j}")
                outproj_pair(j, [0, 1], ob)
                outproj_pair(j, [2, 3], ob)
                nc.scalar.dma_start(out=out_e[j], in_=ob[:, :])

            # ---- schedule ----
            # out-projections are deferred to the end: the PE queue is
            # in-order, so an outproj emitted mid-stream would head-of-line
            # block later projection/attention matmuls whenever its
            # AllGather+readback hasn't landed yet.
            if os.environ.get("KOPT_WARMUP_AG", "1") == "1":
                warmup_ag()
            if os.environ.get("KOPT_HAM_WARMUP", "0") == "1":
                # warms the PE clock gate, but the gate re-demotes during
                # the (variable-length) DMA ramp before the first real
                # chain, so this is net-neutral at best; kept for reference
                ham_warmup()
            if causal:
                proj_sc(0)
                attn_round(0, range(QH))
                ag_fire(0)
                proj_sc(1)
                woc_dma()
                attn_round(1, range(QH))
                ag_fire(1)
                proj_sc(2)
                readback(0)
                attn_round(2, range(QH))
                ag_fire(2)
                proj_sc(3)
                readback(1)
                attn_round(3, range(QH), split=True)
                outproj(0)
                readback(2)
                outproj(1)
                outproj(2)
                readback3()
                outproj3()
            else:
                for sc in range(NSC):
                    proj_sc(sc)
                    if sc == 1:
                        woc_dma()
                attn_round(0, range(QH))
                ag_fire(0)
                attn_round(1, range(QH))
                ag_fire(1)
                readback(0)
                attn_round(2, range(QH))
                ag_fire(2)
                readback(1)
                attn_round(3, range(QH), split=True)
                outproj(0)
                readback(2)
                outproj(1)
                outproj(2)
                readback3()
                outproj3()

    nc.compile()
    return nc


def _perm_cols(w: np.ndarray, heads: list) -> np.ndarray:
    """Reorder head columns to [even dims; odd dims] for block RoPE."""
    cols = []
    for h in heads:
        base = HD * h
        cols.extend([base + 2 * i for i in range(HD // 2)])
        cols.extend([base + 2 * i + 1 for i in range(HD // 2)])
    return np.ascontiguousarray(w[:, cols])


def _pmajor(w: np.ndarray) -> np.ndarray:
    """[2048, N] weight -> [128, 16*N]: one partition's free range is one
    contiguous DRAM run (16 d-blocks side by side)."""
    n = w.shape[1]
    return np.ascontiguousarray(w.reshape(NKT, 128, n).transpose(1, 0, 2)).reshape(
        128, NKT * n
    )


def kernel(x, wq, wk, wv, wo, freqs_cos, freqs_sin, mask):
    from concourse.bass_utils import run_bass_kernel_spmd

    x = np.asarray(x, dtype=np.float32)
    wq = np.asarray(wq, dtype=np.float32)
    wk = np.asarray(wk, dtype=np.float32)
    wv = np.asarray(wv, dtype=np.float32)
    wo = np.asarray(wo, dtype=np.float32)
    freqs_cos = np.asarray(freqs_cos, dtype=np.float32)
    freqs_sin = np.asarray(freqs_sin, dtype=np.float32)
    mask = np.asarray(mask)

    if not mask.any():
        mode = "none"
    elif np.array_equal(mask, np.triu(np.ones((S, S), dtype=bool), k=1)):
        mode = "causal"
    else:
        mode = "general"

    if mode not in _cache:
        import time as _t

        t0 = _t.time()
        _cache[mode] = _build(mode)
        print(f"[kernel] built mode={mode} in {_t.time() - t0:.1f}s", flush=True)
    nc = _cache[mode]

    # ---- host-side prep (sharding + layout) ----
    import ml_dtypes

    # x chunk layout [chunk, p, d-block, s]: per-partition-contiguous
    xt = [
        np.ascontiguousarray(
            np.asarray(x[b].T, dtype=ml_dtypes.bfloat16)
            .reshape(NKT, 128, NSC, SC)
            .transpose(2, 1, 0, 3)
        ).reshape(NSC, 128, NKT * SC)
        for b in range(B)
    ]
    wo_r = wo.astype(ml_dtypes.bfloat16)
    cosT = np.ascontiguousarray(
        np.concatenate([freqs_cos.T, freqs_cos.T], axis=0)
    ).astype(ml_dtypes.bfloat16)
    sinT = np.ascontiguousarray(
        np.concatenate([freqs_sin.T, freqs_sin.T], axis=0)
    ).astype(ml_dtypes.bfloat16)

    if mode == "causal":
        # 4 diag patterns (delta = 0,128,256,384) packed as (128, 2048):
        # bias[i, 512*di + jq] = MASKVAL if (128*di + i) > jq else 0
        i_ = np.arange(HD)[:, None]
        jq = np.arange(SC)[None, :]
        biasd = np.concatenate(
            [
                np.where(128 * di + i_ > jq, np.float32(MASKVAL), np.float32(0.0))
                for di in range(4)
            ],
            axis=1,
        ).astype(np.float32)
    if mode == "general":
        maskb = np.ascontiguousarray(
            np.where(mask.T, np.float32(MASKVAL), np.float32(0.0))
        ).astype(np.float32)

    in_maps = []
    for core in range(NCORES):
        b, g = divmod(core, 4)
        heads = [QH * g + h for h in range(QH)]
        m = {
            "xt": xt[b],
            "wq": _pmajor(_perm_cols(wq, heads)).astype(ml_dtypes.bfloat16),
            "wk": _pmajor(_perm_cols(wk, [g])).astype(ml_dtypes.bfloat16),
            "wv": _pmajor(np.ascontiguousarray(wv[:, HD * g : HD * g + HD])).astype(
                ml_dtypes.bfloat16
            ),
            "woc": _pmajor(
                np.ascontiguousarray(wo_r[:, WOC * g : WOC * g + WOC])
            ),
            "cosT": cosT,
            "sinT": sinT,
        }
        if mode == "causal":
            m["biasd"] = biasd
        if mode == "general":
            m["maskb"] = maskb
        in_maps.append(m)

    import time as _t

    t0 = _t.time()
    print("[kernel] launching SPMD run", flush=True)
    res = run_bass_kernel_spmd(nc, in_maps, core_ids=list(range(NCORES)))
    print(f"[kernel] SPMD run done in {_t.time() - t0:.1f}s", flush=True)
    kernel._last_result = res

    out = np.empty((B, S, DIM), dtype=np.float32)
    for core in range(NCORES):
        b, g = divmod(core, 4)
        # out_e [j, p, st, c] -> rows 512j+128st+p
        o = res.results[core]["out"].reshape(NSC, 128, 4, WOC)
        out[b, :, WOC * g : WOC * g + WOC] = (
            o.transpose(0, 2, 1, 3).reshape(S, WOC).astype(np.float32)
        )
    return out
